# revision 6
# baseline (speedup 1.0000x reference)
"""Trainium2 Bass kernel: two chained SAME-padded 3x3 single-channel convs.

  reference: z = conv3x3(conv3x3(x, w1), w2)   x: [16,1,2048,2048] f32

Strategy (pure data parallel, 2 images per core on 8 cores):
  - Images processed in horizontal bands of S=124 output rows.
  - Each conv is computed on the TensorEngine as 3 banded matmuls
    (one per kernel column dx), accumulating in PSUM. The band matrix
    B_dx[k, m] = W[k-m, dx] applies the vertical taps; the horizontal
    taps come from shifting the moving operand (rhs) by dx columns.
  - conv1 output (y) is copied PSUM->SBUF (VectorE) and consumed by
    conv2's matmuls; conv2 output is copied PSUM->SBUF (ScalarE) and
    DMA'd out. Intermediate y never touches HBM.
  - float32r matmuls (1 cycle/row on the PE vs 4 for fp32; ~1.5e-4
    absmax-relative rounding measured on HW). Set MM_DT = F32 below for
    full fp32 precision at ~4x the PE cost.
  - SAME padding handled with zeroed halo columns in SBUF and
    host-built band-matrix variants for the top/bottom image edges.

Band matrices are built on the host from w1/w2 (they are just 9 floats
each) and passed as extra inputs.
"""

import hashlib
import os
import shutil

import numpy as np

import concourse.mybir as mybir
import concourse.tile as tile
from concourse import bacc, bass2jax
from concourse.bass_utils import run_bass_kernel_spmd


def _install_neff_disk_cache():
    """Cache compiled NEFFs on disk keyed by BIR content hash — the
    neuronxcc backend takes minutes for this kernel and has no cache of
    its own, so a fresh process would otherwise recompile every run."""
    if getattr(bass2jax, "_ant_neff_cache_installed", False):
        return
    orig = bass2jax.compile_bir_kernel

    def cached(bir_json, tmpdir, neff_name="file.neff"):
        try:
            cdir = os.path.expanduser("~/.cache/bass_neff")
            os.makedirs(cdir, exist_ok=True)
            key = hashlib.sha256(
                bir_json if isinstance(bir_json, bytes) else bir_json.encode()
            ).hexdigest()[:32]
            cpath = os.path.join(cdir, f"{key}.neff")
            if os.path.exists(cpath):
                outdir = os.path.join(tmpdir, "sg00")
                os.makedirs(outdir, exist_ok=True)
                dst = os.path.join(outdir, neff_name)
                shutil.copyfile(cpath, dst)
                return dst
            neff = orig(bir_json, tmpdir, neff_name)
            shutil.copyfile(neff, cpath + ".tmp")
            os.replace(cpath + ".tmp", cpath)
            return neff
        except Exception:
            return orig(bir_json, tmpdir, neff_name)

    bass2jax.compile_bir_kernel = cached
    bass2jax._ant_neff_cache_installed = True


_install_neff_disk_cache()

F32 = mybir.dt.float32
F32R = mybir.dt.float32r

MM_DT = F32R  # matmul operand dtype: F32R (fast) or F32 (exact)

NCORES = 8
FULL_B, FULL_H, FULL_W = 16, 2048, 2048

TRACE = False  # set True (from test harness) to capture an NTFF profile
LAST_RESULTS = None  # BassKernelResults of the most recent run


def _build_bands(w1, w2, h, s, nb):
    """Host-side band matrices for the vertical taps.

    B1: [128, 3 variants, 3 dx, 126]; variant 0 = top block, 1 = mid,
    2 = bottom block.  B2: [126, 3 dx, 124].
    """
    W1 = np.asarray(w1, np.float32).reshape(3, 3)
    W2 = np.asarray(w2, np.float32).reshape(3, 3)
    m1, m2 = s + 2, s
    b1 = np.zeros((128, 3, 3, m1), np.float32)
    b2 = np.zeros((m1, 3, m2), np.float32)
    mm = np.arange(m1)
    for i in range(3):
        for dx in range(3):
            b1[mm + i, :, dx, mm] = W1[i, dx]
    mm = np.arange(m2)
    for i in range(3):
        for dx in range(3):
            b2[mm + i, dx, mm] = W2[i, dx]
    # top block: y row r0-1 = -1 is conv2's zero padding, not a computed row
    b1[:, 0, :, 0] = 0.0
    # bottom block: y row == h is zero padding
    r0_last = (nb - 1) * s
    b1[:, 2, :, h - r0_last + 1] = 0.0
    return np.ascontiguousarray(b1.reshape(128, 9 * m1)), np.ascontiguousarray(
        b2.reshape(m1, 3 * m2)
    )


def build_nc(imgs, h, w, nw=512, s=124, repeat=1):
    """Build the per-core Bass program (parametric so a small config can
    be validated in CoreSim)."""
    assert w % nw == 0 and nw <= 512
    nb = -(-h // s)  # blocks per image
    m1, m2 = s + 2, s  # conv1/conv2 output rows per block
    nch = w // nw  # width chunks
    xw = w + 4  # tile width: col 0 zero | 1..w data | w+1 zero | pad
    r0_last = (nb - 1) * s
    rows_last = h - (r0_last - 2)  # x rows loaded for the last block
    k1_last = rows_last + 2

    nc = bacc.Bacc("TRN2", target_bir_lowering=False, debug=False)
    x_d = nc.dram_tensor("x", [imgs, h, w], MM_DT, kind="ExternalInput")
    b1_d = nc.dram_tensor("b1", [128, 9 * m1], MM_DT, kind="ExternalInput")
    b2_d = nc.dram_tensor("b2", [m1, 3 * m2], MM_DT, kind="ExternalInput")
    z_d = nc.dram_tensor("z", [imgs, h, w], F32, kind="ExternalOutput")

    blocks = [(g, b) for g in range(imgs) for b in range(nb)] * repeat

    with tile.TileContext(nc) as tc:
        with (
            tc.tile_pool(name="const", bufs=1) as cpool,
            tc.tile_pool(name="xp", bufs=3) as xpool,
            tc.tile_pool(name="yp", bufs=2) as ypool,
            tc.tile_pool(name="zp", bufs=2) as zpool,
            tc.tile_pool(name="pyp", bufs=4, space="PSUM") as pypool,
            tc.tile_pool(name="pzp", bufs=4, space="PSUM") as pzpool,
        ):
            b1_t = cpool.tile([128, 9 * m1], MM_DT)
            b2_t = cpool.tile([m1, 3 * m2], MM_DT)
            nc.sync.dma_start(out=b1_t[:], in_=b1_d[:])
            nc.sync.dma_start(out=b2_t[:], in_=b2_d[:])

            pend = None  # (img, block, y_tile) awaiting conv2
            for t in range(len(blocks) + 1):
                if t < len(blocks):
                    g, b = blocks[t]
                    r0 = b * s
                    lo, hi = max(r0 - 2, 0), min(r0 + m1, h)
                    p0, rows = lo - (r0 - 2), hi - lo
                    x_t = xpool.tile([128, xw], MM_DT, tag="x")
                    if b == nb - 1:
                        # bottom block: zero the whole tile first (covers the
                        # 2 halo rows below the image and the halo columns);
                        # compute-engine APs can only start at partition
                        # 0/32/64/96, so a targeted halo-row memset is not
                        # expressible.
                        nc.vector.memzero(x_t[:, :])
                    nc.sync.dma_start(
                        out=x_t[p0 : p0 + rows, 1 : 1 + w], in_=x_d[g, lo:hi, :]
                    )
                    if b == 0:
                        nc.vector.memzero(x_t[0:2, :])
                    if b != nb - 1:
                        nc.vector.memzero(x_t[:, 0:1])
                        nc.vector.memzero(x_t[:, 1 + w : 2 + w])
                    k1 = k1_last if b == nb - 1 else 128
                    v = 0 if b == 0 else (2 if b == nb - 1 else 1)
                    y_t = ypool.tile([m1, xw], MM_DT, tag="y")
                    for j in range(nch):
                        py = pypool.tile([m1, nw], F32, tag="py")
                        for dx in range(3):
                            nc.tensor.matmul(
                                py[:],
                                b1_t[0:k1, (v * 3 + dx) * m1 : (v * 3 + dx + 1) * m1],
                                x_t[0:k1, nw * j + dx : nw * j + dx + nw],
                                start=(dx == 0),
                                stop=(dx == 2),
                            )
                        nc.vector.tensor_copy(
                            out=y_t[:, 1 + nw * j : 1 + nw * j + nw], in_=py[:]
                        )
                    nc.vector.memzero(y_t[:, 0:1])
                    nc.vector.memzero(y_t[:, 1 + w : 2 + w])
                    pend_next = (g, b, y_t)
                else:
                    pend_next = None

                if pend is not None:
                    g2, b2i, y_prev = pend
                    r0 = b2i * s
                    rows = min(s, h - r0)
                    z_t = zpool.tile([m2, w], F32, tag="z")
                    for j in range(nch):
                        pz = pzpool.tile([m2, nw], F32, tag="pz")
                        for dx in range(3):
                            nc.tensor.matmul(
                                pz[:],
                                b2_t[0:m1, dx * m2 : (dx + 1) * m2],
                                y_prev[0:m1, nw * j + dx : nw * j + dx + nw],
                                start=(dx == 0),
                                stop=(dx == 2),
                            )
                        nc.scalar.copy(out=z_t[:, nw * j : nw * j + nw], in_=pz[:])
                    nc.sync.dma_start(out=z_d[g2, r0 : r0 + rows, :], in_=z_t[0:rows, :])
                pend = pend_next

    nc.compile()
    return nc


def _build_bands5(w1, w2, h, s, nb):
    """Composite single-pass operator: z = C(x) where C = conv2 o conv1
    with the chained-SAME-padding semantics folded in exactly.

    Vertical behavior (including the y[-1]/y[h] zero rows and the image
    top/bottom) is encoded in per-variant 5-diagonal band matrices
    C[k, v, dx, m].  The only horizontal discrepancy of the composite
    vs the chained convs is the phantom y column at each side; D holds
    the two exact correction bands (applied to x col 0 / w-1, adding
    into z col 0 / w-1).
    """
    W1 = np.asarray(w1, np.float64).reshape(3, 3)
    W2 = np.asarray(w2, np.float64).reshape(3, 3)
    m1, m2 = s + 2, s

    def a_mat(col, rows, cols):
        a = np.zeros((rows, cols), np.float64)
        r = np.arange(rows)
        for i in range(3):
            a[r, r + i] = col[i]
        return a

    r0_last = (nb - 1) * s
    c = np.zeros((128, 3, 5, m2), np.float64)
    d = np.zeros((128, 3, 2, m2), np.float64)
    for v in range(3):
        a1 = [a_mat(W1[:, j], m1, 128) for j in range(3)]
        if v == 0:
            for a in a1:
                a[0, :] = 0.0  # y row -1 is conv2 zero padding
        if v == 2:
            for a in a1:
                a[h - r0_last + 1, :] = 0.0  # y row h is zero padding
        a2 = [a_mat(W2[:, j], m2, m1) for j in range(3)]
        for j in range(3):
            for jp in range(3):
                c[:, v, j + jp, :] += (a2[jp] @ a1[j]).T
        d[:, v, 0, :] = -(a2[0] @ a1[2]).T
        d[:, v, 1, :] = -(a2[2] @ a1[0]).T
    return (
        np.ascontiguousarray(c.reshape(128, 15 * m2).astype(np.float32)),
        np.ascontiguousarray(d.reshape(128, 6 * m2).astype(np.float32)),
    )


def build_nc5(imgs, h, w, nw=512, s=124, repeat=1, xbufs=4, zbufs=3, pzbufs=8, zdma="sync", corr=True):
    """Single-pass composite-5x5 program (see _build_bands5)."""
    assert w % nw == 0 and nw <= 512
    nb = -(-h // s)
    m2 = s
    nch = w // nw
    xw = w + 4  # cols 0,1 zero | 2..w+1 data | w+2,w+3 zero
    r0_last = (nb - 1) * s
    rows_last = h - (r0_last - 2)
    k_last = rows_last + 2

    nc = bacc.Bacc("TRN2", target_bir_lowering=False, debug=False)
    x_d = nc.dram_tensor("x", [imgs, h, w], MM_DT, kind="ExternalInput")
    c_d = nc.dram_tensor("b1", [128, 15 * m2], MM_DT, kind="ExternalInput")
    d_d = nc.dram_tensor("b2", [128, 6 * m2], MM_DT, kind="ExternalInput")
    z_d = nc.dram_tensor("z", [imgs, h, w], F32, kind="ExternalOutput")

    blocks = [(g, b) for g in range(imgs) for b in range(nb)] * repeat

    with tile.TileContext(nc) as tc:
        with (
            tc.tile_pool(name="const", bufs=1) as cpool,
            tc.tile_pool(name="xp", bufs=xbufs) as xpool,
            tc.tile_pool(name="zp", bufs=zbufs) as zpool,
            tc.tile_pool(name="pzp", bufs=pzbufs, space="PSUM") as pzpool,
        ):
            c_t = cpool.tile([128, 15 * m2], MM_DT)
            d_t = cpool.tile([128, 6 * m2], MM_DT)
            nc.sync.dma_start(out=c_t[:], in_=c_d[:])
            nc.sync.dma_start(out=d_t[:], in_=d_d[:])

            for g, b in blocks:
                r0 = b * s
                lo, hi = max(r0 - 2, 0), min(r0 + s + 2, h)
                p0, rows = lo - (r0 - 2), hi - lo
                x_t = xpool.tile([128, xw], MM_DT, tag="x")
                if b == nb - 1:
                    nc.vector.memzero(x_t[:, :])
                nc.sync.dma_start(
                    out=x_t[p0 : p0 + rows, 2 : 2 + w], in_=x_d[g, lo:hi, :]
                )
                if b == 0:
                    nc.vector.memzero(x_t[0:2, :])
                if b != nb - 1:
                    nc.vector.memzero(x_t[:, 0:2])
                    nc.vector.memzero(x_t[:, 2 + w : 4 + w])
                k = k_last if b == nb - 1 else 128
                v = 0 if b == 0 else (2 if b == nb - 1 else 1)
                rows_out = min(s, h - r0)
                z_t = zpool.tile([m2, w], F32, tag="z")
                for j in range(nch):
                    pz = pzpool.tile([m2, nw], F32, tag="pz")
                    corrj = corr and ((j == 0) or (j == nch - 1))
                    for dx in range(5):
                        nc.tensor.matmul(
                            pz[:],
                            c_t[0:k, (v * 5 + dx) * m2 : (v * 5 + dx + 1) * m2],
                            x_t[0:k, nw * j + dx : nw * j + dx + nw],
                            start=(dx == 0),
                            stop=(dx == 4 and not corrj),
                        )
                    # fp32r matmuls need an even moving-operand count and an
                    # 8B-aligned even-count dst, so the 1-column corrections
                    # run as N=2 with the partner column reading a zeroed
                    # halo column of x (negative-step AP) -> contributes 0.
                    if corrj and j == 0:
                        nc.tensor.matmul(
                            pz[:, 0:2],
                            d_t[0:k, (v * 2 + 0) * m2 : (v * 2 + 1) * m2],
                            x_t[0:k, 2:0:-1],  # cols [x 0, zero]
                            start=False,
                            stop=(j != nch - 1),
                        )
                    if corrj and j == nch - 1:
                        nc.tensor.matmul(
                            pz[:, nw - 2 : nw],
                            d_t[0:k, (v * 2 + 1) * m2 : (v * 2 + 2) * m2],
                            x_t[0:k, w + 2 : w : -1],  # cols [zero, x w-1]
                            start=False,
                            stop=True,
                        )
                    if j % 2 == 0:
                        nc.scalar.copy(out=z_t[:, nw * j : nw * j + nw], in_=pz[:])
                    else:
                        nc.vector.tensor_copy(
                            out=z_t[:, nw * j : nw * j + nw], in_=pz[:]
                        )
                zeng = nc.scalar if zdma == "scalar" else nc.sync
                zeng.dma_start(
                    out=z_d[g, r0 : r0 + rows_out, :], in_=z_t[0:rows_out, :]
                )

    nc.compile()
    return nc


def build_nc6(
    imgs, h, w, nw=512, s=124, repeat=1, xbufs=4, zbufs=3, pzbufs=8, zdma="scalar"
):
    """Composite single-pass program over HOST-PADDED x.

    x arrives as [imgs, h+4, w+4] with 2 zero rows/cols on every side,
    so the device needs no halo memsets at all: each band of s output
    rows is one clean [<=128, w+4] DMA, 22 accumulating fp32r matmuls,
    4 PSUM->SBUF copies and one store."""
    assert w % nw == 0 and nw <= 512
    nb = -(-h // s)
    m2 = s
    nch = w // nw
    xw = w + 4
    r0_last = (nb - 1) * s
    k_last = h + 4 - r0_last  # padded rows available for the last block

    nc = bacc.Bacc("TRN2", target_bir_lowering=False, debug=False)
    x_d = nc.dram_tensor("x", [imgs, h + 4, w + 4], MM_DT, kind="ExternalInput")
    c_d = nc.dram_tensor("b1", [128, 15 * m2], MM_DT, kind="ExternalInput")
    d_d = nc.dram_tensor("b2", [128, 6 * m2], MM_DT, kind="ExternalInput")
    z_d = nc.dram_tensor("z", [imgs, h, w], F32, kind="ExternalOutput")

    blocks = [(g, b) for g in range(imgs) for b in range(nb)] * repeat

    with tile.TileContext(nc) as tc:
        with (
            tc.tile_pool(name="const", bufs=1) as cpool,
            tc.tile_pool(name="xp", bufs=xbufs) as xpool,
            tc.tile_pool(name="zp", bufs=zbufs) as zpool,
            tc.tile_pool(name="pzp", bufs=pzbufs, space="PSUM") as pzpool,
        ):
            c_t = cpool.tile([128, 15 * m2], MM_DT)
            d_t = cpool.tile([128, 6 * m2], MM_DT)
            nc.sync.dma_start(out=c_t[:], in_=c_d[:])
            nc.sync.dma_start(out=d_t[:], in_=d_d[:])

            for g, b in blocks:
                r0 = b * s
                k = k_last if b == nb - 1 else 128
                x_t = xpool.tile([128, xw], MM_DT, tag="x")
                nc.sync.dma_start(out=x_t[0:k, :], in_=x_d[g, r0 : r0 + k, :])
                v = 0 if b == 0 else (2 if b == nb - 1 else 1)
                rows_out = min(s, h - r0)
                z_t = zpool.tile([m2, w], F32, tag="z")
                for j in range(nch):
                    pz = pzpool.tile([m2, nw], F32, tag="pz")
                    corrj = (j == 0) or (j == nch - 1)
                    for dx in range(5):
                        nc.tensor.matmul(
                            pz[:],
                            c_t[0:k, (v * 5 + dx) * m2 : (v * 5 + dx + 1) * m2],
                            x_t[0:k, nw * j + dx : nw * j + dx + nw],
                            start=(dx == 0),
                            stop=(dx == 4 and not corrj),
                        )
                    if j == 0:
                        nc.tensor.matmul(
                            pz[:, 0:2],
                            d_t[0:k, (v * 2 + 0) * m2 : (v * 2 + 1) * m2],
                            x_t[0:k, 2:0:-1],  # cols [x 0, zero]
                            start=False,
                            stop=(j != nch - 1),
                        )
                    if j == nch - 1:
                        nc.tensor.matmul(
                            pz[:, nw - 2 : nw],
                            d_t[0:k, (v * 2 + 1) * m2 : (v * 2 + 2) * m2],
                            x_t[0:k, w + 2 : w : -1],  # cols [zero, x w-1]
                            start=False,
                            stop=True,
                        )
                    if j % 2 == 0:
                        nc.scalar.copy(out=z_t[:, nw * j : nw * j + nw], in_=pz[:])
                    else:
                        nc.vector.tensor_copy(
                            out=z_t[:, nw * j : nw * j + nw], in_=pz[:]
                        )
                zeng = nc.scalar if zdma == "scalar" else nc.sync
                zeng.dma_start(
                    out=z_d[g, r0 : r0 + rows_out, :], in_=z_t[0:rows_out, :]
                )

    nc.compile()
    return nc


def pad_x(x, imgs, h, w):
    xp = np.zeros((imgs, h + 4, w + 4), np.float32)
    xp[:, 2 : h + 2, 2 : w + 2] = x
    return xp


BF16 = mybir.dt.bfloat16


def build_nc7(
    imgs, h, w, nw=1024, s=124, repeat=1, xbufs=4, zbufs=3, pzbufs=4, zdma="scalar"
):
    """bf16 end-to-end composite single-pass program over HOST-PADDED x.

    Same math as build_nc6 (composite 5x5 with phantom-column correction
    bands), but x / bands / z are bf16 (PSUM accumulation stays fp32), so
    HBM traffic halves, and the moving operand can be 1024 wide."""
    assert w % nw == 0 and nw <= 1024
    nb = -(-h // s)
    m2 = s
    nch = w // nw
    xw = w + 4
    r0_last = (nb - 1) * s
    k_last = h + 4 - r0_last  # padded rows available for the last block

    nc = bacc.Bacc("TRN2", target_bir_lowering=False, debug=False)
    x_d = nc.dram_tensor("x", [imgs, h + 4, w + 4], BF16, kind="ExternalInput")
    c_d = nc.dram_tensor("b1", [128, 15 * m2], BF16, kind="ExternalInput")
    d_d = nc.dram_tensor("b2", [128, 6 * m2], BF16, kind="ExternalInput")
    z_d = nc.dram_tensor("z", [imgs, h, w], BF16, kind="ExternalOutput")

    blocks = [(g, b) for g in range(imgs) for b in range(nb)] * repeat

    with tile.TileContext(nc) as tc:
        with (
            tc.tile_pool(name="const", bufs=1) as cpool,
            tc.tile_pool(name="xp", bufs=xbufs) as xpool,
            tc.tile_pool(name="zp", bufs=zbufs) as zpool,
            tc.tile_pool(name="pzp", bufs=pzbufs, space="PSUM") as pzpool,
        ):
            c_t = cpool.tile([128, 15 * m2], BF16)
            d_t = cpool.tile([128, 6 * m2], BF16)
            nc.sync.dma_start(out=c_t[:], in_=c_d[:])
            nc.sync.dma_start(out=d_t[:], in_=d_d[:])

            for g, b in blocks:
                r0 = b * s
                k = k_last if b == nb - 1 else 128
                x_t = xpool.tile([128, xw], BF16, tag="x")
                nc.sync.dma_start(out=x_t[0:k, :], in_=x_d[g, r0 : r0 + k, :])
                v = 0 if b == 0 else (2 if b == nb - 1 else 1)
                rows_out = min(s, h - r0)
                z_t = zpool.tile([m2, w], BF16, tag="z")
                for j in range(nch):
                    pz = pzpool.tile([m2, nw], F32, tag="pz")
                    corrj = (j == 0) or (j == nch - 1)
                    for dx in range(5):
                        nc.tensor.matmul(
                            pz[:],
                            c_t[0:k, (v * 5 + dx) * m2 : (v * 5 + dx + 1) * m2],
                            x_t[0:k, nw * j + dx : nw * j + dx + nw],
                            start=(dx == 0),
                            stop=(dx == 4 and not corrj),
                        )
                    if j == 0:
                        nc.tensor.matmul(
                            pz[:, 0:2],
                            d_t[0:k, (v * 2 + 0) * m2 : (v * 2 + 1) * m2],
                            x_t[0:k, 2:0:-1],  # cols [x 0, zero]
                            start=False,
                            stop=(j != nch - 1),
                        )
                    if j == nch - 1:
                        nc.tensor.matmul(
                            pz[:, nw - 2 : nw],
                            d_t[0:k, (v * 2 + 1) * m2 : (v * 2 + 2) * m2],
                            x_t[0:k, w + 2 : w : -1],  # cols [zero, x w-1]
                            start=False,
                            stop=True,
                        )
                    if j % 2 == 0:
                        nc.scalar.copy(out=z_t[:, nw * j : nw * j + nw], in_=pz[:])
                    else:
                        nc.vector.tensor_copy(
                            out=z_t[:, nw * j : nw * j + nw], in_=pz[:]
                        )
                zeng = nc.scalar if zdma == "scalar" else nc.sync
                zeng.dma_start(
                    out=z_d[g, r0 : r0 + rows_out, :], in_=z_t[0:rows_out, :]
                )

    nc.compile()
    return nc


_NC_CACHE = None


def make_in_maps(x, w1, w2):
    """bf16 host prep for build_nc7: pad x by 2 zero px on every side and
    cast to bf16; band matrices cast to bf16."""
    import ml_dtypes

    bf = ml_dtypes.bfloat16
    x = np.asarray(x, np.float32).reshape(FULL_B, FULL_H, FULL_W)
    s, nb = 124, -(-FULL_H // 124)
    c, d = _build_bands5(w1, w2, FULL_H, s, nb)
    c, d = c.astype(bf), d.astype(bf)
    imgs = FULL_B // NCORES
    H, W = FULL_H, FULL_W
    in_maps = []
    for ci in range(NCORES):
        xp = np.zeros((imgs, H + 4, W + 4), bf)
        xp[:, 2 : H + 2, 2 : W + 2] = x[imgs * ci : imgs * (ci + 1)]
        in_maps.append({"x": xp, "b1": c, "b2": d})
    return in_maps


def assemble_output(outs):
    """outs: {'z': [NCORES*imgs, H, W] bf16} concatenated over cores."""
    return np.asarray(outs["z"]).astype(np.float32).reshape(FULL_B, 1, FULL_H, FULL_W)


def get_nc():
    global _NC_CACHE
    if _NC_CACHE is None:
        _NC_CACHE = build_nc7(
            FULL_B // NCORES, FULL_H, FULL_W, nw=512, s=124, pzbufs=8
        )
    return _NC_CACHE


def kernel(x, w1, w2):
    global LAST_RESULTS
    nc = get_nc()
    in_maps = make_in_maps(x, w1, w2)
    res = run_bass_kernel_spmd(nc, in_maps, core_ids=list(range(NCORES)), trace=TRACE)
    LAST_RESULTS = res
    out = np.stack([np.asarray(res.results[c]["z"]) for c in range(NCORES)], axis=0)
    return out.astype(np.float32).reshape(FULL_B, 1, FULL_H, FULL_W)

